# revision 4
# baseline (speedup 1.0000x reference)
"""Trainium2 kernel for FFT-based converged inhibition along the channel axis.

The reference computes y = IFFT(FFT(x, axis=C) / FFT(delta - k_padded)).real,
i.e. a circular convolution of each channel fiber with the fixed length-C
kernel g = IFFT(1/FFT(delta - k)): a circulant matmul Y = G @ X applied at
every (batch, h, w) position.  g decays to <3e-5 beyond +/-32 channels, so
G = I + B with B effectively banded to +/-64.

This kernel computes the residual d = B @ x on device and adds x back on the
host (y = x + d).  That routing keeps the unit diagonal out of the low
precision path, so the tensors crossing HBM can be tiny:
  - x is shipped as fp8e4m3 (error runs only through ||B||~0.14)
  - d returns as absmax-scaled int8
  - the matmul runs in fp8 DoubleRow mode: one 256-deep pass per output
    block using a channel layout shifted by -64 (window [128m-64,128m+192)).
Per-core HBM traffic is ~7 MB vs 25.7 MB for the f32 baseline.

Device strategy (8 NeuronCores, data-parallel over batch): each core gets 2
of the 16 batches.  x arrives pre-rolled by +64 channels so the 4 shifted
128-channel slots are contiguous rows; slot 4 duplicates slot 0 to unwrap
the circular window of the last output block.  Inputs stream per
(batch, column-half) so matmuls start after ~1 MB; 4 matmuls share a
4-bank PSUM quad drained by a single strided DVE/ACT op.
"""

import numpy as np
import ml_dtypes

import concourse.bass as bass
import concourse.tile as tile
from concourse import bacc, mybir
from concourse.bass_utils import run_bass_kernel_spmd

FP8 = ml_dtypes.float8_e4m3  # trn2 float8e4 (IEEE e4m3, max 240)

N_CORES = 8
C = 512
MT = C // 128  # 4 output blocks of 128 channels
NH = 2  # column halves per batch
FCH = 392  # matmul chunk; 4 chunks = one 1568-col half
ALPHA = 128.0  # weight pre-scale so band taps stay in fp8 normal range
X_TARGET = 224.0  # |x|/s_x max; fp8e4 tops out at 240
BETA = 0.25  # d quant range as fraction of max|x| (max|d| ~0.15*max|x|)

_CACHE = {}


def _build_program(n_batch_per_core: int, hw: int, c_drain: float):
    """Per-core SPMD program: d[b] = B @ x[b] (fp8 DoubleRow), d out as int8."""
    ch = hw // NH  # columns per half (1568)
    nf = ch // FCH  # chunks per half (4)
    assert ch * NH == hw and nf * FCH == ch and ch % 16 == 0
    nc = bacc.Bacc(
        "TRN2", target_bir_lowering=False, debug=False, enable_asserts=False
    )
    # x is pre-rolled +64 channels: row 128j+k = original channel 128j-64+k
    x_d = nc.dram_tensor(
        "x", [n_batch_per_core, C, hw], mybir.dt.float8e4, kind="ExternalInput"
    ).ap()
    w_d = nc.dram_tensor(
        "w", [128, MT, 2, 128], mybir.dt.float8e4, kind="ExternalInput"
    ).ap()
    d_d = nc.dram_tensor(
        "d", [n_batch_per_core, C, hw], mybir.dt.int8, kind="ExternalOutput"
    ).ap()

    with tile.TileContext(nc) as tc:
        with (
            tc.tile_pool(name="w", bufs=1) as w_pool,
            tc.tile_pool(name="x", bufs=1) as x_pool,
            tc.tile_pool(name="ps", bufs=2, space="PSUM") as ps_pool,
            tc.tile_pool(name="out", bufs=4) as out_pool,
        ):
            wsb = w_pool.tile([128, MT, 2, 128], mybir.dt.float8e4, tag="w")
            nc.sync.dma_start(wsb[:], w_d)

            # 5 slots per (batch, half): slots 0-3 = shifted 128-ch blocks,
            # slot 4 re-reads slot 0 (unwraps output block 3's window).
            xs = {}
            for b in range(n_batch_per_core):
                for h in range(NH):
                    t = x_pool.tile(
                        [128, 5, ch], mybir.dt.float8e4, tag=f"x{b}_{h}",
                        name=f"x{b}_{h}",
                    )
                    xs[(b, h)] = t
                    for s in range(5):
                        nc.sync.dma_start(
                            t[:, s, :],
                            x_d[
                                b,
                                128 * (s % MT) : 128 * (s % MT) + 128,
                                ch * h : ch * (h + 1),
                            ],
                        )

            # 4 DoubleRow passes (256-deep window) fill a 4-bank PSUM quad,
            # drained int8 by one strided op, alternating DVE/ACT.
            nd = 0
            for b in range(n_batch_per_core):
                for h in range(NH):
                    for m in range(MT):
                        ps = ps_pool.tile(
                            [128, nf, 512], mybir.dt.float32, tag="ps",
                            name=f"ps{b}_{h}_{m}",
                        )
                        for f in range(nf):
                            nc.tensor.matmul(
                                ps[:, f, :FCH],
                                wsb[:, m, :, :],
                                xs[(b, h)][:, m : m + 2, FCH * f : FCH * (f + 1)],
                                start=True,
                                stop=True,
                                perf_mode=mybir.MatmulPerfMode.DoubleRow,
                            )
                        o = out_pool.tile(
                            [128, nf, FCH], mybir.dt.int8, tag="out", name=f"o{nd}"
                        )
                        if nd % 2 == 0:
                            nc.vector.tensor_scalar_mul(o[:], ps[:, :, :FCH], c_drain)
                        else:
                            nc.scalar.mul(o[:], ps[:, :, :FCH], c_drain)
                        nd += 1
                        nc.scalar.dma_start(
                            d_d[b, 128 * m : 128 * (m + 1), ch * h : ch * (h + 1)],
                            o[:],
                        )

    # Hoist the no-wait round-1 input DMA dispatches into the pre-barrier
    # main block: transfers start while the other engines are still in the
    # kernel-entry barrier (~5us earlier).
    try:
        main_blk = nc.main_func.blocks[0]
        sp = mybir.EngineType.SP
        moved = None
        for blk in nc.main_func.blocks[1:]:
            cand = [
                i
                for i in blk.instructions
                if i.engine == sp
                and isinstance(i, mybir.InstDMACopy)
                and not (i.sync_info and i.sync_info.on_wait)
            ]
            if cand:
                moved = cand[:8]
                for i in moved:
                    blk.instructions.remove(i)
                break
        if moved:
            pos = next(
                idx
                for idx, i in enumerate(main_blk.instructions)
                if i.engine == sp and isinstance(i, mybir.InstDrain)
            )
            main_blk.instructions[pos:pos] = moved
    except Exception:
        pass

    # Strip unused const-tile memsets from the preamble (they drag the gpsimd
    # ucode library load into the critical entry barrier).
    for blk in nc.main_func.blocks:
        blk.instructions[:] = [
            inst
            for inst in blk.instructions
            if not (
                isinstance(inst, mybir.InstMemset)
                and inst.outs
                and "const-" in str(inst.outs[0])
            )
        ]
    nc.compile()
    return nc


def _residual_matrix(inhibition_filter: np.ndarray, c: int) -> np.ndarray:
    """B = circulant(g) - I in float64, g = IFFT(1/FFT(delta - pad_roll(k)))."""
    scope = inhibition_filter.shape[0]
    k = np.zeros(c, np.float64)
    k[:scope] = inhibition_filter.astype(np.float64)
    k = np.roll(k, -(scope // 2))
    delta = np.zeros(c, np.float64)
    delta[0] = 1.0
    g = np.fft.ifft(1.0 / np.fft.fft(delta - k)).real
    idx = (np.arange(c)[:, None] - np.arange(c)[None, :]) % c  # G[m, cc] = g[m-cc]
    return g[idx] - np.eye(c)


def _pack_weights(B: np.ndarray) -> np.ndarray:
    """lhsT pack [k, m, j, r] = ALPHA * B[128m+r, (128(m+j)-64+k) % 512]."""
    W = np.zeros((128, MT, 2, 128), np.float64)
    r = np.arange(128)
    kk = np.arange(128)
    for m in range(MT):
        cout = 128 * m + r
        for j in range(2):
            cin = (128 * (m + j) - 64 + kk) % C
            W[:, m, j, :] = ALPHA * B[np.ix_(cout, cin)].T
    return W.astype(FP8)


def _reset_device():
    """Recover a wedged NeuronCore (NRT_EXEC_UNIT_UNRECOVERABLE) via axon."""
    try:
        import ctypes

        import jax

        jax.devices()
        lib = ctypes.CDLL("/opt/axon/libaxon_pjrt.so")
        if hasattr(lib, "axon_reset"):
            lib.axon_reset.restype = ctypes.c_int64
            lib.axon_reset()
    except Exception:
        pass


def kernel(activations: np.ndarray, inhibition_filter: np.ndarray) -> np.ndarray:
    return _run(activations, inhibition_filter, trace=False)[0]


def _run(activations, inhibition_filter, trace=False):
    activations = np.ascontiguousarray(activations, dtype=np.float32)
    n, c, h, w_ = activations.shape
    assert c == C and n % N_CORES == 0
    hw = h * w_
    npc = n // N_CORES

    x = activations.reshape(n, c, hw)
    maxx = float(np.abs(x).max())
    s_x = maxx / X_TARGET
    s_d = BETA * maxx / 127.0
    c_drain = s_x / (ALPHA * s_d)

    B = _residual_matrix(np.asarray(inhibition_filter, np.float32), c)
    wq = _pack_weights(B)

    # roll +64 so shifted slot j = rows [128j, 128j+128) = orig ch 128j-64+k
    xr = np.concatenate([x[:, -64:, :], x[:, :-64, :]], axis=1)
    xq = (xr * (1.0 / s_x)).astype(FP8)
    xq = np.ascontiguousarray(xq.reshape(N_CORES, npc, c, hw))

    key = (npc, hw, round(c_drain, 12))
    if key not in _CACHE:
        _CACHE[key] = _build_program(npc, hw, c_drain)
    nc = _CACHE[key]

    in_maps = [{"x": xq[i], "w": wq} for i in range(N_CORES)]
    try:
        res = run_bass_kernel_spmd(nc, in_maps, list(range(N_CORES)), trace=trace)
    except Exception:
        _reset_device()
        res = run_bass_kernel_spmd(nc, in_maps, list(range(N_CORES)), trace=trace)
    d = np.stack([res.results[i]["d"] for i in range(N_CORES)])
    d = d.reshape(n, c, hw)
    y = x + d.astype(np.float32) * np.float32(s_d)
    return y.reshape(n, c, h, w_).astype(np.float32, copy=False), res
